# revision 5
# baseline (speedup 1.0000x reference)
"""MLA (multi-head latent) causal attention on 8 Trainium2 NeuronCores. v2.

Sharding: batch(4) x head-group(2) mesh over 8 cores. Core c handles batch
c//2 and heads [8*(c%2), 8*(c%2)+8). Latent KV projections are recomputed per
head-group (MLA: latent shared across heads). Each core produces a partial
output (its head-group's contribution to y @ wo^T); the host sums the two
partials per batch.

v2 design (all matmul operands bf16, fp32 PSUM accumulation; ~0.5% rel rms):
  - fused schedule per 512-token chunk n: A(n) projections -> B(n) attention
    over query chunk n -> C(n) output projection, with A(n+1) slices and
    C(n-1) groups interleaved into B(n) to keep the PE dense while ACT does
    the exps.
  - q and y never round-trip DRAM: q_sb/y_sb are double-buffered SBUF tiles.
  - softmax row-sum: per-key-block exps are accumulated into esum on the DVE
    (sliced adds), one small ones-matmul contracts the final 128 partitions.
  - causal mask: Pool affine_select zeroes the upper triangle of the diagonal
    128x128 sub-block of exp (no additive -1e9 pass on the scores).
  - weights (wq, wo, wkv, wku, wvu) and rope tables resident in SBUF, bf16.
"""

import math
from collections import deque
from contextlib import ExitStack

import numpy as np
import ml_dtypes

import concourse.bass as bass
import concourse.mybir as mybir
import concourse.tile as tile
from concourse import bacc
from concourse.bass_utils import run_bass_kernel_spmd

# Problem shape (hardcoded per contract).
B, T, C = 4, 2048, 2048
H, D, L = 16, 128, 512
HG = 8           # heads per core
N_CORES = 8
P = 128
KC = C // P      # 16 contraction chunks over C
LC = L // P      # 4 chunks over L
NQ = T // 512    # 4 query chunks of 512
NT = T // P      # 16 key chunks of 128
SCALE = 1.0 / math.sqrt(D)

F32 = mybir.dt.float32
BF16 = mybir.dt.bfloat16
F8 = mybir.dt.float8e4

_cached = {}


def _build_program():
    nc = bacc.Bacc()

    x8T = nc.dram_tensor("x8T", [C, T], F8, kind="ExternalInput").ap()
    xr8T = nc.dram_tensor("xr8T", [C, T], F8, kind="ExternalInput").ap()
    wq8T = nc.dram_tensor("wq8T", [C, HG * D], F8, kind="ExternalInput").ap()
    wqr8T = nc.dram_tensor("wqr8T", [C, HG * D], F8, kind="ExternalInput").ap()
    wkv8T = nc.dram_tensor("wkv8T", [C, L], F8, kind="ExternalInput").ap()
    wkvr8T = nc.dram_tensor("wkvr8T", [C, L], F8, kind="ExternalInput").ap()
    wkuT = nc.dram_tensor("wkuT", [L, D], BF16, kind="ExternalInput").ap()
    wvuT = nc.dram_tensor("wvuT", [L, D], BF16, kind="ExternalInput").ap()
    woT = nc.dram_tensor("woT", [HG * D, C], BF16, kind="ExternalInput").ap()
    c2 = nc.dram_tensor("c2", [P, T], BF16, kind="ExternalInput").ap()
    s2 = nc.dram_tensor("s2", [P, T], BF16, kind="ExternalInput").ap()
    outp = nc.dram_tensor("outp", [T, C], BF16, kind="ExternalOutput").ap()

    x8T_r = x8T.rearrange("(kcp r p) t -> p kcp r t", p=P, r=2)
    xr8T_r = xr8T.rearrange("(kcp r p) t -> p kcp r t", p=P, r=2)
    wq8T_r = wq8T.rearrange("(kcp r p) e -> p kcp r e", p=P, r=2)
    wqr8T_r = wqr8T.rearrange("(kcp r p) e -> p kcp r e", p=P, r=2)
    wkv8T_r = wkv8T.rearrange("(kcp r p) l -> p kcp r l", p=P, r=2)
    wkvr8T_r = wkvr8T.rearrange("(kcp r p) l -> p kcp r l", p=P, r=2)
    wkuT_r = wkuT.rearrange("(lc p) d -> p lc d", p=P)
    wvuT_r = wvuT.rearrange("(lc p) d -> p lc d", p=P)
    woT_r = woT.rearrange("(h p) c -> p h c", p=P)

    with tile.TileContext(nc) as tc, ExitStack() as top:
        persist = top.enter_context(tc.tile_pool(name="persist", bufs=1))
        pall = top.enter_context(tc.tile_pool(name="pall", bufs=8, space="PSUM"))
        xpool = top.enter_context(tc.tile_pool(name="xpool", bufs=2))
        qpool = top.enter_context(tc.tile_pool(name="qpool", bufs=2))
        ypool = top.enter_context(tc.tile_pool(name="ypool", bufs=2))
        bpool = top.enter_context(tc.tile_pool(name="bpool", bufs=2))
        rpool = top.enter_context(tc.tile_pool(name="rpool", bufs=2))
        ropool = top.enter_context(tc.tile_pool(name="ropool", bufs=1))
        kvpool = top.enter_context(tc.tile_pool(name="kvpool", bufs=1))

        # ---- resident tensors ----
        KCP = KC // 2    # 8 fp8 DoubleRow kc-pair steps
        wkv8_sb = persist.tile([P, KCP, 2, L], F8)       # 8KB/part
        wkvr8_sb = persist.tile([P, KCP, 2, L], F8)
        wq8_sb = persist.tile([P, KCP, 2, HG * P], F8)   # 16KB/part
        wqr8_sb = persist.tile([P, KCP, 2, HG * P], F8)
        wku_sb = persist.tile([P, LC, D], BF16)
        wvu_sb = persist.tile([P, LC, D], BF16)
        ones = persist.tile([P, P], BF16)
        x80 = xpool.tile([P, KCP, 2, 512], F8, tag="x8n", name="x8n0")
        xr80 = xpool.tile([P, KCP, 2, 512], F8, tag="xr8n", name="xr8n0")
        # interleave wkv and x(0) per-kcp slices so chunk-0 kv matmuls can
        # start as soon as the first slices land; the first wq head-pair
        # slice is injected mid-stream so q head 0 never waits on it
        for kcp in range(KCP):
            nc.sync.dma_start(wkv8_sb[:, kcp], wkv8T_r[:, kcp])
            nc.sync.dma_start(x80[:, kcp], x8T_r[:, kcp, :, bass.ts(0, 512)])
            nc.sync.dma_start(wkvr8_sb[:, kcp], wkvr8T_r[:, kcp])
            nc.sync.dma_start(xr80[:, kcp], xr8T_r[:, kcp, :, bass.ts(0, 512)])
            if kcp == 3:
                nc.sync.dma_start(wku_sb[:], wkuT_r)
                nc.sync.dma_start(wvu_sb[:], wvuT_r)
            if kcp == 2:
                nc.sync.dma_start(wq8_sb[:, :, :, bass.ts(0, 2 * P)],
                                  wq8T_r[:, :, :, bass.ts(0, 2 * P)])
                nc.sync.dma_start(wqr8_sb[:, :, :, bass.ts(0, 2 * P)],
                                  wqr8T_r[:, :, :, bass.ts(0, 2 * P)])
        nc.vector.memset(ones[:], 1.0)

        cspool = top.enter_context(tc.tile_pool(name="cspool", bufs=2))

        def fetch_cs(n):
            c2n = cspool.tile([P, 512], BF16, tag="c2n", name=f"c2n{n}")
            s2n = cspool.tile([P, 512], BF16, tag="s2n", name=f"s2n{n}")
            nc.sync.dma_start(c2n[:], c2[:, bass.ts(n, 512)])
            nc.sync.dma_start(s2n[:], s2[:, bass.ts(n, 512)])
            return c2n, s2n

        csns = [fetch_cs(0), None, None, None]

        # remaining wq head-pair column slices, then wo (needed last)
        for g in range(1, 4):
            nc.sync.dma_start(wq8_sb[:, :, :, bass.ts(g, 2 * P)],
                              wq8T_r[:, :, :, bass.ts(g, 2 * P)])
            nc.sync.dma_start(wqr8_sb[:, :, :, bass.ts(g, 2 * P)],
                              wqr8T_r[:, :, :, bass.ts(g, 2 * P)])
        wo_sb = persist.tile([P, HG, C], BF16)           # 32KB/part
        nc.sync.dma_start(wo_sb[:], woT_r)

        # per-chunk k/v slabs (separate tiles so B(n) reads don't serialize
        # against A(n+1) writes)
        k_slabs = [persist.tile([P, 4, P], BF16, name=f"kslab{i}")
                   for i in range(NQ)]
        v_slabs = [persist.tile([P, 4, P], BF16, name=f"vslab{i}")
                   for i in range(NQ)]

        q_sbs = [qpool.tile([P, HG, 512], BF16, tag="qsb", name=f"qsb{i % 2}")
                 for i in range(2)]
        y_sbs = [ypool.tile([P, HG, 512], BF16, tag="ysb", name=f"ysb{i % 2}")
                 for i in range(2)]

        def rope(dst, ps, n, scale=1.0):
            # dst = qq * c2 + swap64(qq) * s2  (per 512-token chunk n)
            c2n, s2n = csns[n]
            qq = ropool.tile([P, 512], BF16, tag="qq")
            qs = ropool.tile([P, 512], BF16, tag="qs")
            m1 = ropool.tile([P, 512], BF16, tag="m1")
            nc.scalar.activation(qq[:], ps[:],
                                 mybir.ActivationFunctionType.Identity,
                                 scale=scale)
            nc.vector.tensor_copy(qs[0:64, :], qq[64:128, :])
            nc.vector.tensor_copy(qs[64:128, :], qq[0:64, :])
            nc.vector.tensor_tensor(m1[:], qq[:], c2n[:],
                                    mybir.AluOpType.mult)
            nc.vector.tensor_tensor(qs[:], qs[:], s2n[:],
                                    mybir.AluOpType.mult)
            nc.vector.tensor_tensor(dst, m1[:], qs[:], mybir.AluOpType.add)

        # ---------- phase A slice emitters ----------
        DR = mybir.MatmulPerfMode.DoubleRow

        def emit_kv(n, x8n, xr8n):
            # latent kv for chunk n: residual-fp8 DoubleRow, 3 passes
            # (x8*w8 + x8*wr8 + xr8*w8), kcp-outer over 4 psum banks;
            # weights are host-scaled x64, compensated in the PSUM copy
            kvps = [pall.tile([P, 512], F32, tag="pa", name=f"kvps{n}_{i}")
                    for i in range(LC)]
            for kcp in range(KCP):
                for pi, (xs, ws) in enumerate(
                        ((x8n, wkv8_sb), (x8n, wkvr8_sb), (xr8n, wkv8_sb))):
                    for lc in range(LC):
                        nc.tensor.matmul(
                            kvps[lc][:],
                            ws[:, kcp, :, bass.ts(lc, P)],
                            xs[:, kcp],
                            start=(kcp == 0 and pi == 0),
                            stop=(kcp == KCP - 1 and pi == 2),
                            perf_mode=DR)
            kvn = kvpool.tile([P, LC, 512], BF16, tag="kvn")
            for lc in range(LC):
                nc.scalar.activation(kvn[:, lc, :], kvps[lc][:],
                                     mybir.ActivationFunctionType.Identity,
                                     scale=1.0 / 64.0)

            kp = pall.tile([P, 512], F32, tag="pa", name=f"kp{n}")
            for lc in range(LC):
                nc.tensor.matmul(kp[:], wku_sb[:, lc, :], kvn[:, lc, :],
                                 start=(lc == 0), stop=(lc == LC - 1))
            kdst = k_slabs[n][:].rearrange("p a b -> p (a b)")
            rope(kdst, kp, n)

            vps = pall.tile([P, 4, P], F32, tag="pa", name=f"vps{n}")
            for i in range(4):
                for lc in range(LC):
                    nc.tensor.matmul(vps[:, i, :],
                                     kvn[:, lc, bass.ts(i, P)],
                                     wvu_sb[:, lc, :],
                                     start=(lc == 0), stop=(lc == LC - 1))
            for i in range(4):
                nc.scalar.copy(v_slabs[n][:, i, :], vps[:, i, :])

        def emit_q_head(n, m, x8n, xr8n):
            # one head's q projection (residual-fp8 DoubleRow) + rope
            qp = pall.tile([P, 512], F32, tag="pa", name=f"qp{n}_{m}")
            for kcp in range(KCP):
                for pi, (xs, ws) in enumerate(
                        ((x8n, wq8_sb), (x8n, wqr8_sb), (xr8n, wq8_sb))):
                    nc.tensor.matmul(qp[:], ws[:, kcp, :, bass.ts(m, P)],
                                     xs[:, kcp],
                                     start=(kcp == 0 and pi == 0),
                                     stop=(kcp == KCP - 1 and pi == 2),
                                     perf_mode=DR)
            rope(q_sbs[n % 2][:, m, :], qp, n, scale=1.0 / 64.0)

        # ---------- phase B per-head emitters ----------
        def emit_scores_chunk(n, h, js, spans, exp_sb):
            q_sb = q_sbs[n % 2]
            for j in js:
                g = spans[j]
                sl = slice(g, 512)
                scp = pall.tile([P, 512], F32, tag="pa",
                                name=f"scp{n}_{h}_{j}")
                nc.tensor.matmul(scp[:, sl], k_slabs[j // 4][:, j % 4, :],
                                 q_sb[:, h, sl], start=True, stop=True)
                nc.scalar.activation(exp_sb[:, j, sl], scp[:, sl],
                                     mybir.ActivationFunctionType.Exp,
                                     scale=SCALE)
                if j >= 4 * n:
                    # zero the in-block upper triangle (q < k) of the exp
                    tri = exp_sb[:, j, g:g + P]
                    nc.gpsimd.affine_select(
                        out=tri, in_=tri,
                        compare_op=mybir.AluOpType.is_ge,
                        fill=0.0, base=0,
                        pattern=[[1, P]], channel_multiplier=-1,
                    )

        def emit_esum_chunk(n, h, js, spans, exp_sb, esum):
            for j in js:
                sl = slice(spans[j], 512)
                if j == 0:
                    nc.vector.tensor_copy(esum[:], exp_sb[:, 0, :])
                else:
                    nc.vector.tensor_tensor(esum[:, sl], esum[:, sl],
                                            exp_sb[:, j, sl],
                                            mybir.AluOpType.add)

        def emit_zpv(n, h, nts, spans, exp_sb, esum):
            zp = pall.tile([P, 512], F32, tag="pa", name=f"zp{n}_{h}")
            nc.tensor.matmul(zp[:], ones[:], esum[:], start=True, stop=True)
            zr = bpool.tile([P, 512], F32, tag="zr")
            nc.vector.reciprocal_approx_fast(out=zr[:], in_=zp[:])

            yp = pall.tile([P, 512], F32, tag="pa", name=f"yp{n}_{h}")
            for j in range(nts):
                sl = slice(spans[j], 512)
                nc.tensor.matmul(yp[:, sl], v_slabs[j // 4][:, j % 4, :],
                                 exp_sb[:, j, sl],
                                 start=(j == 0), stop=(j == nts - 1))
            nc.vector.tensor_tensor(y_sbs[n % 2][:, h, :], yp[:], zr[:],
                                    mybir.AluOpType.mult)

        # ---------- phase C group emitter ----------
        def emit_c_group(n, t16, ci, tail=False):
            y_sb = y_sbs[n % 2]
            ops = pall.tile([P, 512], F32, tag="pa", name=f"ops{n}_{t16}_{ci}")
            for h in range(HG):
                nc.tensor.matmul(ops[:], y_sb[:, h, bass.ts(t16, P)],
                                 wo_sb[:, h, bass.ts(ci, 512)],
                                 start=(h == 0), stop=(h == HG - 1))
            ost = rpool.tile([P, 512], BF16, tag="ost")
            # the tail groups' copies go on ACT, which is idle once the last
            # exps are done, so the DVE isn't the drain bottleneck
            (nc.scalar.copy if tail else nc.vector.tensor_copy)(ost[:], ops[:])
            nc.sync.dma_start(
                outp[bass.ts(4 * n + t16, P), bass.ts(ci, 512)], ost[:])

        # =========================== schedule ===========================
        xns = [(x80, xr80), None, None, None]

        def prefetch_x(n):
            x8n = xpool.tile([P, KCP, 2, 512], F8, tag="x8n", name=f"x8n{n}")
            xr8n = xpool.tile([P, KCP, 2, 512], F8, tag="xr8n", name=f"xr8n{n}")
            nc.sync.dma_start(x8n[:], x8T_r[:, :, :, bass.ts(n, 512)])
            nc.sync.dma_start(xr8n[:], xr8T_r[:, :, :, bass.ts(n, 512)])
            xns[n] = (x8n, xr8n)
            csns[n] = fetch_cs(n)

        # A(0)
        emit_kv(0, x80, xr80)
        for m in range(HG):
            emit_q_head(0, m, x80, xr80)

        for n in range(NQ):
            # fillers emitted inside B(n): C(n-1) groups then A(n+1) slices
            fillers = deque()
            if n > 0:
                for t16 in range(4):
                    for ci in range(4):
                        fillers.append(
                            lambda n=n, t16=t16, ci=ci: emit_c_group(
                                n - 1, t16, ci))
            if n + 1 < NQ:
                prefetch_x(n + 1)
                fillers.append(lambda n=n: emit_kv(n + 1, *xns[n + 1]))
                for m in range(HG):
                    fillers.append(
                        lambda n=n, m=m: emit_q_head(n + 1, m, *xns[n + 1]))

            nts = 4 * (n + 1)
            spans = [max(P * j - 512 * n, 0) for j in range(nts)]
            # budget fillers roughly evenly across the 8 head iterations;
            # software pipeline: z/PV of head h-1 are emitted after the
            # scores+exp of head h, so the PE has dense work while ACT
            # chews through head h's exps
            pending = None
            for h in range(HG):
                exp_sb = bpool.tile([P, NT, 512], BF16, tag="exp")
                esum = bpool.tile([P, 512], BF16, tag="esum")
                quota = (len(fillers) + (HG - h) - 1) // (HG - h)
                js = list(range(nts))
                # emit scores in chunks of 4 blocks, pulling a filler
                # between chunks so the PE never runs >4 psum banks ahead
                # of ACT and always has dense work
                ci = 0
                for c0 in range(0, nts, 4):
                    chunk = js[c0:c0 + 4]
                    emit_scores_chunk(n, h, chunk, spans, exp_sb)
                    emit_esum_chunk(n, h, chunk, spans, exp_sb, esum)
                    if ci < quota and fillers:
                        fillers.popleft()()
                        ci += 1
                if pending is not None:
                    emit_zpv(*pending)
                pending = (n, h, nts, spans, exp_sb, esum)
            emit_zpv(*pending)
            while fillers:
                fillers.popleft()()

        # C(3) tail
        for t16 in range(4):
            for ci in range(4):
                emit_c_group(NQ - 1, t16, ci, tail=(t16 >= 2))

    nc.finalize()
    return nc


_PERM = np.concatenate([np.arange(0, D, 2), np.arange(1, D, 2)])
_BF = ml_dtypes.bfloat16
_F8 = ml_dtypes.float8_e4m3
_WSCALE = 64.0


def _split8(a):
    # residual fp8 split: a ~= a8 + ar8
    a = np.ascontiguousarray(a, dtype=np.float32)
    a8 = a.astype(_F8)
    ar8 = (a - a8.astype(np.float32)).astype(_F8)
    return a8, ar8


def _prep_core_inputs(x, freqs_cos, freqs_sin, wq, wkv_down, wk_up, wv_up, wo):
    cosT = np.ascontiguousarray(freqs_cos.T).astype(np.float32)   # [64, T]
    sinT = np.ascontiguousarray(freqs_sin.T).astype(np.float32)
    c2 = np.concatenate([cosT, cosT], axis=0).astype(_BF)         # [128, T]
    s2 = np.concatenate([-sinT, sinT], axis=0).astype(_BF)

    wkv8, wkvr8 = _split8(wkv_down.T * _WSCALE)                   # [C, L]
    wkuT = np.ascontiguousarray(wk_up[_PERM, :].T).astype(_BF)    # [L, D]
    wvuT = np.ascontiguousarray(wv_up.T).astype(_BF)              # [L, D]

    wq_h = wq.reshape(H, D, C)[:, _PERM, :]                       # perm rows/head

    in_maps = []
    for core in range(N_CORES):
        b, g = core // 2, core % 2
        heads = slice(8 * g, 8 * g + 8)
        wq8, wqr8 = _split8(
            wq_h[heads].reshape(HG * D, C).T * _WSCALE)           # [C, 1024]
        woT_g = np.ascontiguousarray(
            wo[:, 8 * g * D:(8 * g + 8) * D].T).astype(_BF)       # [1024, C]
        x8, xr8 = _split8(x[b].T)                                 # [C, T]
        in_maps.append({
            "x8T": x8, "xr8T": xr8, "wq8T": wq8, "wqr8T": wqr8,
            "wkv8T": wkv8, "wkvr8T": wkvr8, "wkuT": wkuT,
            "wvuT": wvuT, "woT": woT_g, "c2": c2, "s2": s2,
        })
    return in_maps


def kernel(x, freqs_cos, freqs_sin, wq, wkv_down, wk_up, wv_up, wo, _trace=False):
    x = np.asarray(x, dtype=np.float32)
    freqs_cos = np.asarray(freqs_cos, dtype=np.float32)
    freqs_sin = np.asarray(freqs_sin, dtype=np.float32)
    wq = np.asarray(wq, dtype=np.float32)
    wkv_down = np.asarray(wkv_down, dtype=np.float32)
    wk_up = np.asarray(wk_up, dtype=np.float32)
    wv_up = np.asarray(wv_up, dtype=np.float32)
    wo = np.asarray(wo, dtype=np.float32)

    if "nc" not in _cached:
        _cached["nc"] = _build_program()
    nc = _cached["nc"]

    in_maps = _prep_core_inputs(x, freqs_cos, freqs_sin, wq, wkv_down,
                                wk_up, wv_up, wo)
    res = run_bass_kernel_spmd(nc, in_maps, core_ids=list(range(N_CORES)),
                               trace=_trace)
    _cached["last_result"] = res

    out = np.empty((B, T, C), dtype=np.float32)
    for b in range(B):
        out[b] = res.results[2 * b]["outp"] + res.results[2 * b + 1]["outp"]
    return out


# revision 7
# speedup vs baseline: 1.1890x; 1.1890x over previous
"""MLA (multi-head latent) causal attention on 8 Trainium2 NeuronCores. v2.

Sharding: batch(4) x head-group(2) mesh over 8 cores. Core c handles batch
c//2 and heads [8*(c%2), 8*(c%2)+8). Latent KV projections are recomputed per
head-group (MLA: latent shared across heads). Each core produces a partial
output (its head-group's contribution to y @ wo^T); the host sums the two
partials per batch.

v2 design (all matmul operands bf16, fp32 PSUM accumulation; ~0.5% rel rms):
  - fused schedule per 512-token chunk n: A(n) projections -> B(n) attention
    over query chunk n -> C(n) output projection, with A(n+1) slices and
    C(n-1) groups interleaved into B(n) to keep the PE dense while ACT does
    the exps.
  - q and y never round-trip DRAM: q_sb/y_sb are double-buffered SBUF tiles.
  - softmax row-sum: per-key-block exps are accumulated into esum on the DVE
    (sliced adds), one small ones-matmul contracts the final 128 partitions.
  - causal mask: Pool affine_select zeroes the upper triangle of the diagonal
    128x128 sub-block of exp (no additive -1e9 pass on the scores).
  - weights (wq, wo, wkv, wku, wvu) and rope tables resident in SBUF, bf16.
"""

import math
from collections import deque
from contextlib import ExitStack

import numpy as np
import ml_dtypes

import concourse.bass as bass
import concourse.mybir as mybir
import concourse.tile as tile
from concourse import bacc
from concourse.bass_utils import run_bass_kernel_spmd

# Problem shape (hardcoded per contract).
B, T, C = 4, 2048, 2048
H, D, L = 16, 128, 512
HG = 8           # heads per core
N_CORES = 8
P = 128
KC = C // P      # 16 contraction chunks over C
LC = L // P      # 4 chunks over L
NQ = T // 512    # 4 query chunks of 512
NT = T // P      # 16 key chunks of 128
SCALE = 1.0 / math.sqrt(D)

F32 = mybir.dt.float32
BF16 = mybir.dt.bfloat16

SIM_NOCC = False   # analyze.py sets True: replace collectives with local DMA

_cached = {}


def _build_program():
    nc = bacc.Bacc()

    xT = nc.dram_tensor("xT", [C, T], BF16, kind="ExternalInput").ap()
    xkvT = nc.dram_tensor("xkvT", [C, NQ * 256], BF16, kind="ExternalInput").ap()
    c2k = nc.dram_tensor("c2k", [P, NQ * 256], BF16, kind="ExternalInput").ap()
    s2k = nc.dram_tensor("s2k", [P, NQ * 256], BF16, kind="ExternalInput").ap()
    stgT = nc.dram_tensor("stgT", [NQ, P, 512], BF16, kind="Internal").ap()
    gatT = nc.dram_tensor("gatT", [NQ, 2, P, 512], BF16, kind="Internal").ap()
    wqT = nc.dram_tensor("wqT", [C, HG * D], BF16, kind="ExternalInput").ap()
    wkvT = nc.dram_tensor("wkvT", [C, L], BF16, kind="ExternalInput").ap()
    wkuT = nc.dram_tensor("wkuT", [L, D], BF16, kind="ExternalInput").ap()
    wvuT = nc.dram_tensor("wvuT", [L, D], BF16, kind="ExternalInput").ap()
    woT = nc.dram_tensor("woT", [HG * D, C], BF16, kind="ExternalInput").ap()
    c2 = nc.dram_tensor("c2", [P, T], BF16, kind="ExternalInput").ap()
    s2 = nc.dram_tensor("s2", [P, T], BF16, kind="ExternalInput").ap()
    outp = nc.dram_tensor("outp", [T, C], BF16, kind="ExternalOutput").ap()

    xT_r = xT.rearrange("(kc p) t -> p kc t", p=P)
    xkvT_r = xkvT.rearrange("(kc p) t -> p kc t", p=P)
    wqT_r = wqT.rearrange("(kc p) e -> p kc e", p=P)
    wkvT_r = wkvT.rearrange("(kc p) l -> p kc l", p=P)
    wkuT_r = wkuT.rearrange("(lc p) d -> p lc d", p=P)
    wvuT_r = wvuT.rearrange("(lc p) d -> p lc d", p=P)
    woT_r = woT.rearrange("(h p) c -> p h c", p=P)

    with tile.TileContext(nc) as tc, ExitStack() as top:
        persist = top.enter_context(tc.tile_pool(name="persist", bufs=1))
        pall = top.enter_context(tc.tile_pool(name="pall", bufs=8, space="PSUM"))
        xpool = top.enter_context(tc.tile_pool(name="xpool", bufs=2))
        qpool = top.enter_context(tc.tile_pool(name="qpool", bufs=2))
        ypool = top.enter_context(tc.tile_pool(name="ypool", bufs=2))
        bpool = top.enter_context(tc.tile_pool(name="bpool", bufs=2))
        rpool = top.enter_context(tc.tile_pool(name="rpool", bufs=2))
        ropool = top.enter_context(tc.tile_pool(name="ropool", bufs=1))
        kvpool = top.enter_context(tc.tile_pool(name="kvpool", bufs=1))

        # ---- resident tensors ----
        wq_sb = persist.tile([P, KC, HG * P], BF16)      # 32KB/part
        wku_sb = persist.tile([P, LC, D], BF16)
        wvu_sb = persist.tile([P, LC, D], BF16)
        ones = persist.tile([P, P], BF16)
        wkv_sb = persist.tile([P, KC, L], BF16)          # 16KB/part
        xkv0 = xpool.tile([P, KC, 256], BF16, tag="xkv", name="xkv0")
        x0 = xpool.tile([P, KC, 512], BF16, tag="xn", name="xn0", bufs=1)
        # interleave wkv and xkv (kv-half tokens) in 4-kc groups so chunk-0
        # kv matmuls start after the first pair of DMAs lands; x(0) for the
        # q projections and the first wq head-pair slice follow
        for g4 in range(4):
            sl = bass.ts(g4, 4)
            nc.sync.dma_start(wkv_sb[:, sl, :], wkvT_r[:, sl, :])
            nc.sync.dma_start(xkv0[:, sl, :], xkvT_r[:, sl, bass.ts(0, 256)])
            if g4 == 0:
                nc.sync.dma_start(wku_sb[:], wkuT_r)
                nc.sync.dma_start(wvu_sb[:], wvuT_r)
            if g4 == 2:
                nc.sync.dma_start(wq_sb[:, :, bass.ts(0, 2 * P)],
                                  wqT_r[:, :, bass.ts(0, 2 * P)])
        nc.sync.dma_start(x0[:], xT_r[:, :, bass.ts(0, 512)])
        nc.vector.memset(ones[:], 1.0)

        cspool = top.enter_context(tc.tile_pool(name="cspool", bufs=2))

        def fetch_cs(n):
            c2n = cspool.tile([P, 512], BF16, tag="c2n", name=f"c2n{n}")
            s2n = cspool.tile([P, 512], BF16, tag="s2n", name=f"s2n{n}")
            c2kn = ropool.tile([P, 256], BF16, tag="c2kn", name=f"c2kn{n}")
            s2kn = ropool.tile([P, 256], BF16, tag="s2kn", name=f"s2kn{n}")
            nc.sync.dma_start(c2kn[:], c2k[:, bass.ts(n, 256)])
            nc.sync.dma_start(s2kn[:], s2k[:, bass.ts(n, 256)])
            nc.sync.dma_start(c2n[:], c2[:, bass.ts(n, 512)])
            nc.sync.dma_start(s2n[:], s2[:, bass.ts(n, 512)])
            return c2n, s2n, c2kn, s2kn

        csns = [fetch_cs(0), None, None, None]

        wo_sb = persist.tile([P, HG, C], BF16)           # 32KB/part

        # per-chunk k/v slabs (separate tiles so B(n) reads don't serialize
        # against A(n+1) writes)
        k_slabs = [persist.tile([P, 4, P], BF16, name=f"kslab{i}")
                   for i in range(NQ)]
        v_slabs = [persist.tile([P, 4, P], BF16, name=f"vslab{i}")
                   for i in range(NQ)]

        q_sbs = [qpool.tile([P, HG, 512], BF16, tag="qsb", name=f"qsb{i % 2}")
                 for i in range(2)]
        y_sbs = [ypool.tile([P, HG, 512], BF16, tag="ysb", name=f"ysb{i % 2}")
                 for i in range(2)]

        def rope(dst, ps, c2a, s2a, w):
            # dst = qq * c2 + swap64(qq) * s2  ([128, w] tiles)
            qq = ropool.tile([P, 512], BF16, tag="qq", name="qq")
            qs = ropool.tile([P, 512], BF16, tag="qs", name="qs")
            m1 = ropool.tile([P, 512], BF16, tag="m1", name="m1")
            qq, qs, m1 = qq[:, 0:w], qs[:, 0:w], m1[:, 0:w]
            nc.scalar.copy(qq[:], ps[:])
            nc.vector.tensor_copy(qs[0:64, :], qq[64:128, :])
            nc.vector.tensor_copy(qs[64:128, :], qq[0:64, :])
            nc.vector.tensor_tensor(m1[:], qq[:], c2a,
                                    mybir.AluOpType.mult)
            nc.vector.tensor_tensor(qs[:], qs[:], s2a,
                                    mybir.AluOpType.mult)
            nc.vector.tensor_tensor(dst, m1[:], qs[:], mybir.AluOpType.add)

        # ---------- phase A slice emitters ----------
        def emit_kv(n, xkv):
            # latent kv for THIS RANK'S 256-token half of chunk n (the host
            # gives even cores the first half, odd cores the second half,
            # with matching rope tables), then k-up+rope and v; the halves
            # are exchanged pairwise via AllGather through HBM and the
            # full chunk's k/v slabs are read back in true token order
            kvps = [pall.tile([P, 256], F32, tag="pa", name=f"kvps{n}_{i}")
                    for i in range(LC)]
            for kc in range(KC):
                for lc in range(LC):
                    nc.tensor.matmul(kvps[lc][:],
                                     wkv_sb[:, kc, bass.ts(lc, P)],
                                     xkv[:, kc, :],
                                     start=(kc == 0), stop=(kc == KC - 1))
            kvn = kvpool.tile([P, LC, 256], BF16, tag="kvn")
            for lc in range(LC):
                nc.scalar.copy(kvn[:, lc, :], kvps[lc][:])

            kp = pall.tile([P, 256], F32, tag="pa", name=f"kp{n}")
            for lc in range(LC):
                nc.tensor.matmul(kp[:], wku_sb[:, lc, :], kvn[:, lc, :],
                                 start=(lc == 0), stop=(lc == LC - 1))
            kvh = kvpool.tile([P, 512], BF16, tag="kvh")
            c2kn, s2kn = csns[n][2], csns[n][3]
            rope(kvh[:, 0:256], kp, c2kn[:], s2kn[:], 256)

            vps = pall.tile([P, 2, P], F32, tag="pa", name=f"vps{n}")
            for i in range(2):
                for lc in range(LC):
                    nc.tensor.matmul(vps[:, i, :],
                                     kvn[:, lc, bass.ts(i, P)],
                                     wvu_sb[:, lc, :],
                                     start=(lc == 0), stop=(lc == LC - 1))
                nc.scalar.copy(kvh[:, 256 + 128 * i:384 + 128 * i],
                               vps[:, i, :])

            # stage -> pairwise AllGather -> read back full chunk
            nc.sync.dma_start(stgT[n], kvh[:])
            if SIM_NOCC:
                nc.sync.dma_start(gatT[n, 0], stgT[n])
                nc.sync.dma_start(gatT[n, 1], stgT[n])
            else:
                nc.gpsimd.collective_compute(
                    "AllGather", mybir.AluOpType.bypass,
                    replica_groups=[[0, 1], [2, 3], [4, 5], [6, 7]],
                    ins=[stgT[n]], outs=[gatT[n]],
                )
            nc.sync.dma_start(
                k_slabs[n][:],
                gatT[n, :, :, 0:256].rearrange("r p (b t) -> p r b t", b=2))
            nc.sync.dma_start(
                v_slabs[n][:],
                gatT[n, :, :, 256:512].rearrange("r p (b d) -> p r b d", b=2))

        def emit_q_head(n, m, xn):
            # one head's q projection + rope into q_sbs[n % 2]
            qp = pall.tile([P, 512], F32, tag="pa", name=f"qp{n}_{m}")
            for kc in range(KC):
                nc.tensor.matmul(qp[:], wq_sb[:, kc, bass.ts(m, P)],
                                 xn[:, kc, :],
                                 start=(kc == 0), stop=(kc == KC - 1))
            c2n, s2n = csns[n][0], csns[n][1]
            rope(q_sbs[n % 2][:, m, :], qp, c2n[:], s2n[:], 512)

        # ---------- phase B per-head emitters ----------
        def emit_scores_chunk(n, h, js, spans, exp_sb):
            q_sb = q_sbs[n % 2]
            for j in js:
                g = spans[j]
                sl = slice(g, 512)
                scp = pall.tile([P, 512], F32, tag="pa",
                                name=f"scp{n}_{h}_{j}")
                nc.tensor.matmul(scp[:, sl], k_slabs[j // 4][:, j % 4, :],
                                 q_sb[:, h, sl], start=True, stop=True)
                nc.scalar.activation(exp_sb[:, j, sl], scp[:, sl],
                                     mybir.ActivationFunctionType.Exp,
                                     scale=SCALE)
                if j >= 4 * n:
                    # zero the in-block upper triangle (q < k) of the exp
                    tri = exp_sb[:, j, g:g + P]
                    nc.gpsimd.affine_select(
                        out=tri, in_=tri,
                        compare_op=mybir.AluOpType.is_ge,
                        fill=0.0, base=0,
                        pattern=[[1, P]], channel_multiplier=-1,
                    )

        def emit_esum_chunk(n, h, js, spans, exp_sb, esum):
            for j in js:
                sl = slice(spans[j], 512)
                if j == 0:
                    nc.vector.tensor_copy(esum[:], exp_sb[:, 0, :])
                else:
                    nc.vector.tensor_tensor(esum[:, sl], esum[:, sl],
                                            exp_sb[:, j, sl],
                                            mybir.AluOpType.add)

        def emit_zpv(n, h, nts, spans, exp_sb, esum):
            zp = pall.tile([P, 512], F32, tag="pa", name=f"zp{n}_{h}")
            nc.tensor.matmul(zp[:], ones[:], esum[:], start=True, stop=True)
            zr = bpool.tile([P, 512], F32, tag="zr")
            nc.vector.reciprocal_approx_fast(out=zr[:], in_=zp[:])

            yp = pall.tile([P, 512], F32, tag="pa", name=f"yp{n}_{h}")
            for j in range(nts):
                sl = slice(spans[j], 512)
                nc.tensor.matmul(yp[:, sl], v_slabs[j // 4][:, j % 4, :],
                                 exp_sb[:, j, sl],
                                 start=(j == 0), stop=(j == nts - 1))
            nc.vector.tensor_tensor(y_sbs[n % 2][:, h, :], yp[:], zr[:],
                                    mybir.AluOpType.mult)

        # ---------- phase C group emitter ----------
        def emit_c_group(n, t16, ci, tail=False):
            y_sb = y_sbs[n % 2]
            ops = pall.tile([P, 512], F32, tag="pa", name=f"ops{n}_{t16}_{ci}")
            for h in range(HG):
                nc.tensor.matmul(ops[:], y_sb[:, h, bass.ts(t16, P)],
                                 wo_sb[:, h, bass.ts(ci, 512)],
                                 start=(h == 0), stop=(h == HG - 1))
            ost = rpool.tile([P, 512], BF16, tag="ost", name="ost")
            # the tail groups' copies go on ACT, which is idle once the last
            # exps are done, so the DVE isn't the drain bottleneck
            (nc.scalar.copy if tail else nc.vector.tensor_copy)(ost[:], ops[:])
            nc.sync.dma_start(
                outp[bass.ts(4 * n + t16, P), bass.ts(ci, 512)], ost[:])

        # =========================== schedule ===========================
        xns = [x0, None, None, None]

        xkvs = [xkv0, None, None, None]

        def prefetch_xkv(n):
            xkv = xpool.tile([P, KC, 256], BF16, tag="xkv", name=f"xkv{n}")
            nc.sync.dma_start(xkv[:], xkvT_r[:, :, bass.ts(n, 256)])
            xkvs[n] = xkv
            csns[n] = fetch_cs(n)

        def fetch_xn(n):
            xn = xpool.tile([P, KC, 512], BF16, tag="xn", name=f"xn{n}", bufs=1)
            nc.sync.dma_start(xn[:], xT_r[:, :, bass.ts(n, 512)])
            xns[n] = xn

        # A(0); the remaining wq head-pair slices stream in behind the
        # chunk-0 kv weight tiles, wo (first needed by C(0) during B(1))
        # after the q weights
        emit_kv(0, xkv0)
        for g in range(1, 4):
            nc.sync.dma_start(wq_sb[:, :, bass.ts(g, 2 * P)],
                              wqT_r[:, :, bass.ts(g, 2 * P)])
        for m in range(HG):
            emit_q_head(0, m, x0)
        nc.sync.dma_start(wo_sb[:], woT_r)

        for n in range(NQ):
            # fillers emitted inside B(n): C(n-1) groups then A(n+1) slices
            fillers = deque()
            cgroups = deque()
            if n > 0:
                for t16 in range(4):
                    for ci in range(4):
                        cgroups.append(
                            lambda n=n, t16=t16, ci=ci: emit_c_group(
                                n - 1, t16, ci))
            # order: a few C groups, then kv(n+1) (its streamed weight
            # tiles must not queue behind the big xn prefetch), then the
            # xn prefetch, then the rest
            for _ in range(4):
                if cgroups:
                    fillers.append(cgroups.popleft())
            if n + 1 < NQ:
                prefetch_xkv(n + 1)
                fillers.append(lambda n=n: emit_kv(n + 1, xkvs[n + 1]))
                fillers.append(lambda n=n: fetch_xn(n + 1))
            qheads = deque()
            if n + 1 < NQ:
                for m in range(HG):
                    qheads.append(
                        lambda n=n, m=m: emit_q_head(n + 1, m, xns[n + 1]))
            while cgroups or qheads:
                if cgroups:
                    fillers.append(cgroups.popleft())
                if qheads:
                    fillers.append(qheads.popleft())

            nts = 4 * (n + 1)
            spans = [max(P * j - 512 * n, 0) for j in range(nts)]
            # budget fillers roughly evenly across the 8 head iterations;
            # software pipeline: z/PV of head h-1 are emitted after the
            # scores+exp of head h, so the PE has dense work while ACT
            # chews through head h's exps
            pending = None
            for h in range(HG):
                exp_sb = bpool.tile([P, NT, 512], BF16, tag="exp")
                esum = bpool.tile([P, 512], BF16, tag="esum")
                quota = (len(fillers) + (HG - h) - 1) // (HG - h)
                js = list(range(nts))
                # emit scores in chunks of 4 blocks, pulling a filler
                # between chunks so the PE never runs >4 psum banks ahead
                # of ACT and always has dense work
                ci = 0
                for c0 in range(0, nts, 4):
                    chunk = js[c0:c0 + 4]
                    emit_scores_chunk(n, h, chunk, spans, exp_sb)
                    emit_esum_chunk(n, h, chunk, spans, exp_sb, esum)
                    if ci < quota and fillers:
                        fillers.popleft()()
                        ci += 1
                if pending is not None:
                    emit_zpv(*pending)
                pending = (n, h, nts, spans, exp_sb, esum)
            emit_zpv(*pending)
            while fillers:
                fillers.popleft()()

        # C(3) tail
        for t16 in range(4):
            for ci in range(4):
                emit_c_group(NQ - 1, t16, ci, tail=(t16 >= 2))

    nc.finalize()
    return nc


_PERM = np.concatenate([np.arange(0, D, 2), np.arange(1, D, 2)])
_BF = ml_dtypes.bfloat16


def _prep_core_inputs(x, freqs_cos, freqs_sin, wq, wkv_down, wk_up, wv_up, wo):
    cosT = np.ascontiguousarray(freqs_cos.T).astype(np.float32)   # [64, T]
    sinT = np.ascontiguousarray(freqs_sin.T).astype(np.float32)
    c2 = np.concatenate([cosT, cosT], axis=0).astype(_BF)         # [128, T]
    s2 = np.concatenate([-sinT, sinT], axis=0).astype(_BF)

    wkvT = np.ascontiguousarray(wkv_down.T).astype(_BF)           # [C, L]
    wkuT = np.ascontiguousarray(wk_up[_PERM, :].T).astype(_BF)    # [L, D]
    wvuT = np.ascontiguousarray(wv_up.T).astype(_BF)              # [L, D]

    wq_h = wq.reshape(H, D, C)[:, _PERM, :]                       # perm rows/head

    c2f = np.concatenate([cosT, cosT], axis=0)                    # [128, T] f32
    s2f = np.concatenate([-sinT, sinT], axis=0)
    in_maps = []
    for core in range(N_CORES):
        b, g = core // 2, core % 2
        r = core % 2     # rank within the batch pair: token-half of each chunk
        heads = slice(8 * g, 8 * g + 8)
        wqT_g = np.ascontiguousarray(
            wq_h[heads].reshape(HG * D, C).T).astype(_BF)         # [C, 1024]
        woT_g = np.ascontiguousarray(
            wo[:, 8 * g * D:(8 * g + 8) * D].T).astype(_BF)       # [1024, C]
        xT_b = np.ascontiguousarray(x[b].T).astype(_BF)           # [C, T]
        cols = np.concatenate(
            [np.arange(512 * n + 256 * r, 512 * n + 256 * r + 256)
             for n in range(NQ)])
        xkv = np.ascontiguousarray(xT_b[:, cols])                 # [C, 1024]
        c2k_ = np.ascontiguousarray(c2f[:, cols]).astype(_BF)
        s2k_ = np.ascontiguousarray(s2f[:, cols]).astype(_BF)
        in_maps.append({
            "xT": xT_b, "xkvT": xkv, "wqT": wqT_g, "wkvT": wkvT,
            "wkuT": wkuT, "wvuT": wvuT, "woT": woT_g, "c2": c2, "s2": s2,
            "c2k": c2k_, "s2k": s2k_,
        })
    return in_maps


def kernel(x, freqs_cos, freqs_sin, wq, wkv_down, wk_up, wv_up, wo, _trace=False):
    x = np.asarray(x, dtype=np.float32)
    freqs_cos = np.asarray(freqs_cos, dtype=np.float32)
    freqs_sin = np.asarray(freqs_sin, dtype=np.float32)
    wq = np.asarray(wq, dtype=np.float32)
    wkv_down = np.asarray(wkv_down, dtype=np.float32)
    wk_up = np.asarray(wk_up, dtype=np.float32)
    wv_up = np.asarray(wv_up, dtype=np.float32)
    wo = np.asarray(wo, dtype=np.float32)

    if "nc" not in _cached:
        _cached["nc"] = _build_program()
    nc = _cached["nc"]

    in_maps = _prep_core_inputs(x, freqs_cos, freqs_sin, wq, wkv_down,
                                wk_up, wv_up, wo)
    res = run_bass_kernel_spmd(nc, in_maps, core_ids=list(range(N_CORES)),
                               trace=_trace)
    _cached["last_result"] = res

    out = np.empty((B, T, C), dtype=np.float32)
    for b in range(B):
        out[b] = res.results[2 * b]["outp"] + res.results[2 * b + 1]["outp"]
    return out
